# revision 46
# baseline (speedup 1.0000x reference)
"""GCN layer kernel for Trainium2: out[b] = D^-1/2 (A[b]+I) D^-1/2 H[b] B.

Data-parallel, one graph per NeuronCore, no collectives.

Precision/decomposition strategy (host prep is O(N^2) scaling plus the tiny
O(N F O) feature product — 1.5% of the FLOPs; the device runs the dominant
O(N^2 O) product as a single PE-bound streaming pass):
    A = 0.5·J + A',  A' = A - 0.5 ∈ [-0.5, 0.5)
    d = 1/sqrt(1 + rowsum(A))                       (host)
    X = D H B                                       (host, bf16, 0.5MB)
Device computes (one pass over the fp8 A' stream):
    Z[o,i] = sum_t X_t^T @ A'T_t[:,i]  +  B^T (dH)^T[:,i]
Host finishes with the rank-1 mean term and the left normalization:
    w[o] = 0.5·(d @ H) @ B
    out[i,o] = d_i · (Z[o,i] + w[o])

A' ships UNSCALED in fp8 E3M4 (TRN FP8_EXP3): for values in [-0.5, 0.5) it
is a uniform ~6-bit quantizer (abs step 2^-6), and the quantization noise
averages over the 2048-term contraction (~9e-3 max rel vs the 2e-2 gate).
Mean-centering halves the top-binade error; mixed bf16-stationary x
fp8-moving matmuls are exact on HW. HBM traffic ~4.8MB/core.

Layout/stream notes (from trace analysis):
 - A' pre-slabbed on host to [128, 16*2048] so each chunk DMA is one
   contiguous 2-8KB run per partition; everything on the Sync HWDGE queue
   (the Scalar queue measured ~144 GB/s vs Sync's ~425 here).
 - Tapered chunks [1,1,2,4,4,4]: DMA semaphores fire ~1.46x the transfer
   time apart (completion-path serialization, lag ~3us at stream end), a
   cadence that matches the PE's 0.86us/slab pace; few chunks minimize
   sem overhead, the fine head starts the PE early.
 - Head DMA ships bw|X only (0.56MB) so the mains start ~10.4us; (dH)^T
   ships second-to-last since only the stop-pass needs it.
 - Dummy matmuls on a memset tile warm the HAM clock gate during the
   pre-data idle so real matmuls run at 2.4GHz (512-col MM every 215ns)
   from the first slab.
 - The +I term (B^T (dH)^T) is the stop-pass: 4 matmuls close the 4
   independent one-bank PSUM accumulators, each block then evacuates
   (DVE cast to bf16) and DMAs out immediately.
Output leaves as bf16 [O, N]; host adds w, scales by d_i, upcasts,
transposes.
"""
import sys

sys.path.insert(0, "/opt/trn_rl_repo")

import numpy as np
import ml_dtypes

BF16 = ml_dtypes.bfloat16
FP8 = ml_dtypes.float8_e3m4
B_, N_, F_, O_ = 8, 2048, 128, 128
NT = N_ // 128  # 16 slabs
# Tapered chunk sizes: DMA semaphores fire ~1.4x the transfer time apart
# (completion-path serialization), which matches the PE's 0.86us/slab pace;
# few chunks keep sem overhead low, the tapered head starts the PE early.
STREAM_GROUPS = [[0], [1], [2, 3], [4, 5, 6, 7], [8, 9, 10, 11], [12, 13, 14, 15]]
PE_ORDER = list(range(16))
N_CORES = 8

_CACHE = {}
LAST_RESULTS = None


def _build_program():
    import concourse.bacc as bacc
    import concourse.tile as tile
    import concourse.mybir as mybir

    f32 = mybir.dt.float32
    bf16 = mybir.dt.bfloat16
    fp8 = mybir.dt.float8e3

    nc = bacc.Bacc(None, target_bir_lowering=False)
    # packed: [p, t*N_+i] = A'T[t*128+p, i], fp8 e3m4
    ATS = nc.dram_tensor("ats", [128, NT * N_], fp8, kind="ExternalInput")
    # bw | xt  (xt packed: [p, t*128+o] = X[t*128+p, o])
    HH = nc.dram_tensor("hh", [F_, 128 + N_], bf16, kind="ExternalInput")
    # (d ⊙ H)^T for the +I stop-pass; ships mid-stream
    DHT = nc.dram_tensor("dht", [F_, N_], bf16, kind="ExternalInput")
    OT = nc.dram_tensor("ot", [O_, N_], bf16, kind="ExternalOutput")

    with tile.TileContext(nc) as tc:
        with (
            tc.tile_pool(name="const", bufs=1) as cst,
            tc.tile_pool(name="achunks", bufs=1) as ach,
            tc.tile_pool(name="outp", bufs=4) as outp,
            tc.tile_pool(name="psbig", bufs=1, space="PSUM") as psb,
        ):
            # HAM warmup: hold the PE clock at 2.4GHz while the head DMA
            # streams in (sized to end right as slab 0's semaphore fires)
            wu_sb = cst.tile([128, 512], bf16, tag="wu")
            nc.vector.memset(wu_sb, 0.0)
            wu_ps = psb.tile([128, 512], f32, tag="wu")
            for _ in range(14):
                nc.tensor.matmul(wu_ps, wu_sb[:, 0:128], wu_sb, start=True, stop=True)

            hh_sb = cst.tile([128, 128 + N_], bf16, tag="hh")
            nc.sync.dma_start(out=hh_sb, in_=HH[:, :])
            bw = hh_sb[:, 0:128]
            xs = [hh_sb[:, 128 + t * 128 : 128 + (t + 1) * 128] for t in range(NT)]
            dht_sb = cst.tile([128, N_], bf16, tag="dht")

            # A' chunks on the Sync HWDGE ring in stream order; dht slots
            # in before the last chunk (the stop-pass needs it at the end)
            at_slab = [None] * NT
            for ci, grp in enumerate(STREAM_GROUPS):
                if ci == len(STREAM_GROUPS) - 1:
                    nc.sync.dma_start(out=dht_sb, in_=DHT[:, :])
                st, csz = grp[0], len(grp)
                assert grp == list(range(st, st + csz))
                t = ach.tile([128, csz * N_], fp8, tag=f"at{ci}")
                nc.sync.dma_start(out=t, in_=ATS[:, st * N_ : (st + csz) * N_])
                for sl in range(csz):
                    at_slab[st + sl] = t[:, sl * N_ : (sl + 1) * N_]

            # 4 independent one-bank accumulators for Z^T's 512-col blocks
            yt = []
            for b in range(4):
                yt_b = psb.tile([128, 512], f32, tag=f"yt{b}")
                yt.append(yt_b)

            # main accumulation: one pass over the fp8 A' stream, in PE_ORDER
            for k, t in enumerate(PE_ORDER):
                for b in range(4):
                    nc.tensor.matmul(
                        yt[b],
                        xs[t],
                        at_slab[t][:, b * 512 : (b + 1) * 512],
                        start=(k == 0),
                        stop=False,
                    )
            # +I self-loop term B^T (dH)^T closes each block (stop=True),
            # then evacuate + write out as bf16 (DVE only: an ACT-copy
            # variant for the last block measured ~6us slower)
            for b in range(4):
                nc.tensor.matmul(
                    yt[b],
                    bw,
                    dht_sb[:, b * 512 : (b + 1) * 512],
                    start=False,
                    stop=True,
                )
                ost = outp.tile([128, 512], bf16, tag="ost")
                nc.vector.tensor_copy(ost, yt[b])
                nc.sync.dma_start(out=OT[:, b * 512 : (b + 1) * 512], in_=ost)

    nc.compile()
    return nc


def _get_program():
    if "nc" not in _CACHE:
        _CACHE["nc"] = _build_program()
    return _CACHE["nc"]


def kernel(H, A, B):
    global LAST_RESULTS
    from concourse.bass_utils import run_bass_kernel_spmd

    nc = _get_program()

    H32 = np.asarray(H, dtype=np.float32)
    A32 = np.asarray(A, dtype=np.float32)
    B32 = np.asarray(B, dtype=np.float32)

    in_maps = []
    ws = []
    ds = []
    for b in range(B_):
        Ab = A32[b]
        dvec = (1.0 / np.sqrt(1.0 + Ab.sum(axis=1, dtype=np.float64))).astype(
            np.float32
        )
        ds.append(dvec)
        # centered, unscaled A' in fp8 e3m4, slab-packed transposed
        ats_packed = (
            np.ascontiguousarray((Ab - 0.5).T.reshape(NT, 128, N_).transpose(1, 0, 2))
            .reshape(128, NT * N_)
            .astype(FP8)
        )
        Hb = H32[b]
        # X = D H B (fp32 product, one bf16 rounding), slab-packed
        X = dvec[:, None] * (Hb @ B32)
        hh = np.empty((F_, 128 + N_), dtype=BF16)
        hh[:, 0:128] = B32.astype(BF16)
        hh[:, 128:] = (
            np.ascontiguousarray(X.reshape(NT, 128, O_).transpose(1, 0, 2))
            .reshape(128, NT * O_)
            .astype(BF16)
        )
        dht = np.ascontiguousarray((Hb * dvec[:, None]).T).astype(BF16)
        in_maps.append({"ats": ats_packed, "hh": hh, "dht": dht})
        # host rank-1 mean term: w[o] = 0.5 * (d @ H) @ B
        ws.append(0.5 * (dvec.astype(np.float64) @ Hb.astype(np.float64)) @ B32)

    res = run_bass_kernel_spmd(nc, in_maps, list(range(N_CORES)))
    LAST_RESULTS = res

    out = np.empty((B_, N_, O_), dtype=np.float32)
    for b in range(B_):
        zt = res.results[b]["ot"].astype(np.float32)  # [O, N]
        out[b] = (zt + ws[b].astype(np.float32)[:, None]).T * ds[b][:, None]
    return out
